# revision 28
# baseline (speedup 1.0000x reference)
"""MoE layer (switch-style top-1 routing) on 8 Trainium2 NeuronCores.

Strategy (expert-parallel, matching the layer's original dist.all_to_all
dispatch): the host computes the tiny top-1 router (0.06% of total FLOPs,
in fp64 so argmax decisions are exact), then dispatches each expert's
tokens to the core that owns that expert -- the host plays the role of the
all-to-all fabric since kernel() receives full inputs.  Each core runs a
dense 2-matmul FFN (x @ w1 -> relu -> @ w2) for its single expert in bf16
with fp32 PSUM accumulation, in a token-transposed layout ([D,T] /
[F,T]) so the contraction dim sits on SBUF partitions and biases are
per-partition scalars.  The host scatters results back and applies the
router-prob scaling.

All weight/activation tensors are packed host-side into the exact SBUF
tile layout so every DMA is a single fully-contiguous transfer.
"""

import math
import os
import sys

import numpy as np

sys.path.insert(0, "/opt/trn_rl_repo")

import ml_dtypes  # noqa: E402


def _install_ntff_hook_shim():
    """The container's antenv stub lacks axon_hooks, which silently disables
    NTFF profiling (trace=True) in run_bass_kernel_spmd.  Recreate the module
    and wire it to the boot-time ctypes hook so traces work."""
    try:
        import antenv.axon_hooks  # noqa: F401
        return
    except ImportError:
        pass
    try:
        import types

        import antenv
        from trn_agent_boot.trn_boot import _ntff_profile_via_ctypes

        hook = _ntff_profile_via_ctypes("/opt/axon/libaxon_pjrt.so")
        mod = types.ModuleType("antenv.axon_hooks")
        mod._hook = hook
        mod.get_axon_ntff_profile_hook = lambda: mod._hook
        mod.set_axon_ntff_profile_hook = lambda h: setattr(mod, "_hook", h)
        sys.modules["antenv.axon_hooks"] = mod
        antenv.axon_hooks = mod
    except Exception:
        pass


_install_ntff_hook_shim()

import concourse.bass as bass  # noqa: E402
import concourse.mybir as mybir  # noqa: E402
import concourse.tile as tile  # noqa: E402
from concourse import bacc  # noqa: E402
from concourse.bass_utils import run_bass_kernel_spmd  # noqa: E402

B, S, D, F, E = 4, 1024, 768, 3072, 8
N_CORES = 8
KD = D // 128  # 6 contraction tiles over D
KF = F // 128  # 24 tiles over F
# w1 DMA group sizes (in f-tiles of 196KB): small first group so the PE can
# start as soon as xt + group 0 land; later groups big enough to keep the
# HBM stream efficient and ahead of consumption.
W1_GROUPS = [2, 2, 3, 3, 3, 3, 3, 3, 2]
assert sum(W1_GROUPS) == KF
BF16 = mybir.dt.bfloat16
F32 = mybir.dt.float32
BF16_NP = ml_dtypes.bfloat16

_build_cache: dict = {}
last_results = None  # BassKernelResults of the most recent run (for test.py)


def build_bass(chunks: tuple) -> bass.Bass:
    """One-expert FFN over sum(chunks) tokens, one PSUM-sized chunk at a time."""
    Tpad = sum(chunks)
    assert all(c <= 512 and c % 8 == 0 for c in chunks)
    # Bacc (not plain Bass): its compile() runs move_matmul_waits_to_ldweights
    # + generate_event_semaphores, which this walrus requires (max 1 sync wait
    # per engine instruction).
    nc = bacc.Bacc()
    # Host packs weights in the exact SBUF layout, f/d-tile-major, so every
    # DMA group is a fully contiguous per-partition range of columns.
    xt = nc.dram_tensor("xt", [128, KD, Tpad], BF16, kind="ExternalInput")
    w1 = nc.dram_tensor("w1", [128, KF * KD * 128], BF16, kind="ExternalInput")
    w2 = nc.dram_tensor("w2", [128, KD * KF * 128], BF16, kind="ExternalInput")
    b1 = nc.dram_tensor("b1", [128, KF], F32, kind="ExternalInput")
    b2 = nc.dram_tensor("b2", [128, KD], F32, kind="ExternalInput")
    yt = nc.dram_tensor("yt", [128, KD, Tpad], F32, kind="ExternalOutput")

    with tile.TileContext(nc) as tc:
        with (
            tc.tile_pool(name="wpool", bufs=1) as wpool,
            tc.tile_pool(name="cpool", bufs=1) as cpool,
            tc.tile_pool(name="hpool", bufs=KF + 2) as hpool,
            tc.tile_pool(name="ypool", bufs=min(len(chunks) * KD, 8)) as ypool,
            tc.tile_pool(name="pspool", bufs=7, space=bass.MemorySpace.PSUM) as pspool,
        ):
            w1_t = [
                wpool.tile([128, nf, KD, 128], BF16, tag=f"w1g{g}", name=f"w1t{g}")
                for g, nf in enumerate(W1_GROUPS)
            ]
            w2_t = [
                wpool.tile([128, KF, 128], BF16, tag=f"w2d{d}", name=f"w2t{d}")
                for d in range(KD)
            ]
            xt_c = [
                cpool.tile([128, KD, Tc], BF16, tag=f"xt{c}", name=f"xt_c{c}")
                for c, Tc in enumerate(chunks)
            ]
            b1_t = cpool.tile([128, KF], F32, tag="b1", name="b1_t")
            b2_t = cpool.tile([128, KD], F32, tag="b2", name="b2_t")

            # Inputs split across BOTH HWDGE rings (each ring is FIFO, both
            # stream concurrently): weights on the SP ring in consumption
            # order; tokens + biases on the ACT ring.  The first matmul needs
            # only w1 group 0 (SP) + chunk-0 tokens (ACT), which now download
            # in parallel.  Output stores also ride the ACT ring, emitted
            # after all its inputs.
            chunk_off = []
            off = 0
            for Tc in chunks:
                chunk_off.append(off)
                off += Tc
            f0 = 0
            for g, nf in enumerate(W1_GROUPS):
                nc.sync.dma_start(
                    out=w1_t[g][:],
                    in_=w1[:, f0 * KD * 128:(f0 + nf) * KD * 128],
                )
                f0 += nf
            for d in range(KD):
                nc.sync.dma_start(
                    out=w2_t[d][:], in_=w2[:, d * KF * 128:(d + 1) * KF * 128]
                )
            for c in range(len(chunks)):
                nc.scalar.dma_start(
                    out=xt_c[c][:],
                    in_=xt[:, :, chunk_off[c]:chunk_off[c] + chunks[c]],
                )
            nc.scalar.dma_start(out=b1_t[:], in_=b1[:])
            nc.scalar.dma_start(out=b2_t[:], in_=b2[:])

            # PE warm-up: ~4.5us of dummy matmuls with no DMA dependency, so
            # the HAM clock gate opens (1.2 -> 2.4 GHz) while the input DMAs
            # are still streaming.  They finish before the first real weights
            # arrive, so they cost no wall-clock.
            warm = cpool.tile([128, 512], BF16, tag="warm", name="warm")
            warm_ps = pspool.tile([64, 512], F32, tag="warmps", name="warm_ps", bufs=1)
            nc.gpsimd.memset(warm[:], 0.0)
            for i in range(13):
                nc.tensor.matmul(
                    warm_ps[:], warm[:, :64], warm[:], start=True, stop=True
                )

            # f-tile index -> (dma group, index within group)
            f_loc = []
            for g, nf in enumerate(W1_GROUPS):
                f_loc += [(g, i) for i in range(nf)]

            for c, Tc in enumerate(chunks):
                tok = slice(chunk_off[c], chunk_off[c] + Tc)
                h_tiles = []
                # h[f,t] = relu(b1[f] + sum_k w1[k,f] * x[k,t]), f-tile at a time
                for f in range(KF):
                    g, fi = f_loc[f]
                    ps = pspool.tile([128, Tc], F32, tag="ps", name=f"ps1_{c}_{f}")
                    for k in range(KD):
                        nc.tensor.matmul(
                            ps[:],
                            w1_t[g][:, fi, k, :],
                            xt_c[c][:, k, :],
                            start=(k == 0),
                            stop=(k == KD - 1),
                        )
                    h = hpool.tile([128, Tc], BF16, tag="h", name=f"h_{c}_{f}")
                    nc.scalar.activation(
                        h[:], ps[:], mybir.ActivationFunctionType.Relu,
                        bias=b1_t[:, f:f + 1], scale=1.0,
                    )
                    h_tiles.append(h)
                # y[d,t] = b2[d] + sum_f w2[f,d] * h[f,t]
                for d in range(KD):
                    ps2 = pspool.tile([128, Tc], F32, tag="ps", name=f"ps2_{c}_{d}")
                    for f in range(KF):
                        nc.tensor.matmul(
                            ps2[:],
                            w2_t[d][:, f, :],
                            h_tiles[f][:],
                            start=(f == 0),
                            stop=(f == KF - 1),
                        )
                    yo = ypool.tile([128, Tc], F32, tag="y", name=f"y_{c}_{d}")
                    nc.scalar.activation(
                        yo[:], ps2[:], mybir.ActivationFunctionType.Identity,
                        bias=b2_t[:, d:d + 1], scale=1.0,
                    )
                    nc.scalar.dma_start(out=yt[:, d, tok], in_=yo[:])
    nc.finalize()
    return nc


def _chunking(Tmax: int):
    """Balanced chunk sizes (multiples of 8, each <=512) covering Tmax."""
    Tmax = max(Tmax, 16)
    nch = max(1, math.ceil(Tmax / 512))
    units = math.ceil(Tmax / 8)
    per, extra = divmod(units, nch)
    chunks = tuple(8 * (per + (1 if i < extra else 0)) for i in range(nch))
    return chunks


def route_host(hidden_states, w_router):
    """fp64 router: logits, top-1 index, top-1 softmax prob."""
    hs_flat = np.asarray(hidden_states, np.float64).reshape(-1, D)
    logits = hs_flat @ np.asarray(w_router, np.float64)
    eidx = logits.argmax(1).astype(np.int32)
    ex = np.exp(logits - logits.max(1, keepdims=True))
    ptop = ex.max(1) / ex.sum(1)
    return logits, eidx, ptop


def pack_core_inputs(hs_flat32, tok, w1e, b1e, w2e, b2e, Tpad):
    """Pack one expert's tokens + weights into the device tile layouts."""
    n_e = len(tok)
    xt_h = np.zeros((128, KD, Tpad), BF16_NP)
    if n_e:
        xe = hs_flat32[tok]  # [n_e, D]
        xt_h[:, :, :n_e] = np.ascontiguousarray(
            xe.T.reshape(KD, 128, n_e).transpose(1, 0, 2)
        ).astype(BF16_NP)
    # w1 [D,F] -> [p, f, k, c] (f-tile-major columns), flattened per partition
    w1h = (
        w1e.reshape(KD, 128, KF, 128)
        .transpose(1, 2, 0, 3)
        .reshape(128, KF * KD * 128)
        .astype(BF16_NP)
    )
    # w2 [F,D] -> [p, d, kf, c] (d-tile-major columns)
    w2h = (
        w2e.reshape(KF, 128, KD, 128)
        .transpose(1, 2, 0, 3)
        .reshape(128, KD * KF * 128)
        .astype(BF16_NP)
    )
    b1h = np.ascontiguousarray(b1e.reshape(KF, 128).T).astype(np.float32)
    b2h = np.ascontiguousarray(b2e.reshape(KD, 128).T).astype(np.float32)
    return {"xt": xt_h, "w1": w1h, "w2": w2h, "b1": b1h, "b2": b2h}


def kernel(hidden_states, w_router, w1, b1, w2, b2):
    global last_results
    hs_flat32 = np.asarray(hidden_states, np.float32).reshape(-1, D)

    logits64, eidx, ptop = route_host(hidden_states, w_router)
    router_logits = logits64.astype(np.float32).reshape(B, S, E)

    counts = np.bincount(eidx, minlength=E)
    order = np.argsort(eidx, kind="stable")
    bounds = np.concatenate([[0], np.cumsum(counts)])
    chunks = _chunking(int(counts.max()))
    Tpad = sum(chunks)

    if chunks not in _build_cache:
        _build_cache[chunks] = build_bass(chunks)
    nc = _build_cache[chunks]

    w1 = np.asarray(w1, np.float32)
    w2 = np.asarray(w2, np.float32)
    b1 = np.asarray(b1, np.float32)
    b2 = np.asarray(b2, np.float32)
    toks = [order[bounds[e]:bounds[e + 1]] for e in range(E)]
    in_maps = [
        pack_core_inputs(hs_flat32, toks[e], w1[e], b1[e], w2[e], b2[e], Tpad)
        for e in range(E)
    ]

    trace = os.environ.get("BASS_KERNEL_TRACE") == "1"
    try:
        last_results = run_bass_kernel_spmd(
            nc, in_maps, core_ids=list(range(N_CORES)), trace=trace,
        )
    except Exception:
        # transient device/runtime hiccups happen on the shared machine;
        # one retry before giving up
        last_results = run_bass_kernel_spmd(
            nc, in_maps, core_ids=list(range(N_CORES)), trace=trace,
        )

    out_flat = np.zeros((B * S, D), np.float32)
    for e in range(E):
        n_e = int(counts[e])
        if n_e == 0:
            continue
        yt_r = last_results.results[e]["yt"]  # [128, KD, Tpad] f32
        ye = yt_r.transpose(1, 0, 2).reshape(D, Tpad)[:, :n_e].T  # [n_e, D]
        out_flat[toks[e]] = ptop[toks[e], None].astype(np.float32) * ye

    return (
        out_flat.reshape(B, S, D),
        router_logits,
        eidx.reshape(B, S),
    )


# revision 29
# speedup vs baseline: 1.0801x; 1.0801x over previous
"""MoE layer (switch-style top-1 routing) on 8 Trainium2 NeuronCores.

Strategy (expert-parallel, matching the layer's original dist.all_to_all
dispatch): the host computes the tiny top-1 router (0.06% of total FLOPs,
in fp64 so argmax decisions are exact), then dispatches each expert's
tokens to the core that owns that expert -- the host plays the role of the
all-to-all fabric since kernel() receives full inputs.  Each core runs a
dense 2-matmul FFN (x @ w1 -> relu -> @ w2) for its single expert in bf16
with fp32 PSUM accumulation, in a token-transposed layout ([D,T] /
[F,T]) so the contraction dim sits on SBUF partitions and biases are
per-partition scalars.  The host scatters results back and applies the
router-prob scaling.

All weight/activation tensors are packed host-side into the exact SBUF
tile layout so every DMA is a single fully-contiguous transfer.
"""

import math
import os
import sys

import numpy as np

sys.path.insert(0, "/opt/trn_rl_repo")

import ml_dtypes  # noqa: E402


def _install_ntff_hook_shim():
    """The container's antenv stub lacks axon_hooks, which silently disables
    NTFF profiling (trace=True) in run_bass_kernel_spmd.  Recreate the module
    and wire it to the boot-time ctypes hook so traces work."""
    try:
        import antenv.axon_hooks  # noqa: F401
        return
    except ImportError:
        pass
    try:
        import types

        import antenv
        from trn_agent_boot.trn_boot import _ntff_profile_via_ctypes

        hook = _ntff_profile_via_ctypes("/opt/axon/libaxon_pjrt.so")
        mod = types.ModuleType("antenv.axon_hooks")
        mod._hook = hook
        mod.get_axon_ntff_profile_hook = lambda: mod._hook
        mod.set_axon_ntff_profile_hook = lambda h: setattr(mod, "_hook", h)
        sys.modules["antenv.axon_hooks"] = mod
        antenv.axon_hooks = mod
    except Exception:
        pass


_install_ntff_hook_shim()

import concourse.bass as bass  # noqa: E402
import concourse.mybir as mybir  # noqa: E402
import concourse.tile as tile  # noqa: E402
from concourse import bacc  # noqa: E402
from concourse.bass_utils import run_bass_kernel_spmd  # noqa: E402

B, S, D, F, E = 4, 1024, 768, 3072, 8
N_CORES = 8
KD = D // 128  # 6 contraction tiles over D
KF = F // 128  # 24 tiles over F
# w1 DMA group sizes (in f-tiles of 196KB): small first group so the PE can
# start as soon as xt + group 0 land; later groups big enough to keep the
# HBM stream efficient and ahead of consumption.
W1_GROUPS = [2, 2, 3, 3, 3, 3, 3, 3, 2]
assert sum(W1_GROUPS) == KF
BF16 = mybir.dt.bfloat16
F32 = mybir.dt.float32
BF16_NP = ml_dtypes.bfloat16

_build_cache: dict = {}
last_results = None  # BassKernelResults of the most recent run (for test.py)


def build_bass(chunks: tuple) -> bass.Bass:
    """One-expert FFN over sum(chunks) tokens, one PSUM-sized chunk at a time."""
    Tpad = sum(chunks)
    assert all(c <= 512 and c % 8 == 0 for c in chunks)
    # Bacc (not plain Bass): its compile() runs move_matmul_waits_to_ldweights
    # + generate_event_semaphores, which this walrus requires (max 1 sync wait
    # per engine instruction).
    nc = bacc.Bacc()
    # Host packs weights in the exact SBUF layout, f/d-tile-major, so every
    # DMA group is a fully contiguous per-partition range of columns.
    xt = nc.dram_tensor("xt", [128, KD, Tpad], BF16, kind="ExternalInput")
    w1 = nc.dram_tensor("w1", [128, KF * KD * 128], BF16, kind="ExternalInput")
    w2 = nc.dram_tensor("w2", [128, KD * KF * 128], BF16, kind="ExternalInput")
    b1 = nc.dram_tensor("b1", [128, KF], F32, kind="ExternalInput")
    b2 = nc.dram_tensor("b2", [128, KD], F32, kind="ExternalInput")
    yt = nc.dram_tensor("yt", [128, KD, Tpad], F32, kind="ExternalOutput")

    with tile.TileContext(nc) as tc:
        with (
            tc.tile_pool(name="wpool", bufs=1) as wpool,
            tc.tile_pool(name="cpool", bufs=1) as cpool,
            tc.tile_pool(name="hpool", bufs=KF + 2) as hpool,
            tc.tile_pool(name="ypool", bufs=min(len(chunks) * KD, 8)) as ypool,
            tc.tile_pool(name="pspool", bufs=7, space=bass.MemorySpace.PSUM) as pspool,
        ):
            w1_t = [
                wpool.tile([128, nf, KD, 128], BF16, tag=f"w1g{g}", name=f"w1t{g}")
                for g, nf in enumerate(W1_GROUPS)
            ]
            w2_t = [
                wpool.tile([128, KF, 128], BF16, tag=f"w2d{d}", name=f"w2t{d}")
                for d in range(KD)
            ]
            xt_c = [
                cpool.tile([128, KD, Tc], BF16, tag=f"xt{c}", name=f"xt_c{c}")
                for c, Tc in enumerate(chunks)
            ]
            b1_t = cpool.tile([128, KF], F32, tag="b1", name="b1_t")
            b2_t = cpool.tile([128, KD], F32, tag="b2", name="b2_t")

            # Input stream on the SP HWDGE ring (FIFO), in consumption order:
            # w1 group 0 + chunk-0 tokens first (first matmul group), the rest
            # of w1, then w2, then the later token chunks.  (Splitting inputs
            # across both rings was tried and is ~7us slower: the rings share
            # HBM bandwidth, so the w1 stream falls behind its consumption.)
            chunk_off = []
            off = 0
            for Tc in chunks:
                chunk_off.append(off)
                off += Tc
            nc.sync.dma_start(
                out=w1_t[0][:], in_=w1[:, :W1_GROUPS[0] * KD * 128]
            )
            nc.sync.dma_start(
                out=xt_c[0][:], in_=xt[:, :, chunk_off[0]:chunk_off[0] + chunks[0]]
            )
            f0 = W1_GROUPS[0]
            for g, nf in list(enumerate(W1_GROUPS))[1:]:
                nc.sync.dma_start(
                    out=w1_t[g][:],
                    in_=w1[:, f0 * KD * 128:(f0 + nf) * KD * 128],
                )
                f0 += nf
            for d in range(KD):
                nc.sync.dma_start(
                    out=w2_t[d][:], in_=w2[:, d * KF * 128:(d + 1) * KF * 128]
                )
            for c in range(1, len(chunks)):
                nc.sync.dma_start(
                    out=xt_c[c][:],
                    in_=xt[:, :, chunk_off[c]:chunk_off[c] + chunks[c]],
                )
            # Biases (tiny) + output stores ride the ACT HWDGE ring so they
            # never queue behind the bulk input stream.
            nc.scalar.dma_start(out=b1_t[:], in_=b1[:])
            nc.scalar.dma_start(out=b2_t[:], in_=b2[:])

            # PE warm-up: ~4.5us of dummy matmuls with no DMA dependency, so
            # the HAM clock gate opens (1.2 -> 2.4 GHz) while the input DMAs
            # are still streaming.  They finish before the first real weights
            # arrive, so they cost no wall-clock.
            warm = cpool.tile([128, 512], BF16, tag="warm", name="warm")
            warm_ps = pspool.tile([64, 512], F32, tag="warmps", name="warm_ps", bufs=1)
            nc.gpsimd.memset(warm[:], 0.0)
            for i in range(13):
                nc.tensor.matmul(
                    warm_ps[:], warm[:, :64], warm[:], start=True, stop=True
                )

            # f-tile index -> (dma group, index within group)
            f_loc = []
            for g, nf in enumerate(W1_GROUPS):
                f_loc += [(g, i) for i in range(nf)]

            for c, Tc in enumerate(chunks):
                tok = slice(chunk_off[c], chunk_off[c] + Tc)
                h_tiles = []
                # h[f,t] = relu(b1[f] + sum_k w1[k,f] * x[k,t]), f-tile at a time
                for f in range(KF):
                    g, fi = f_loc[f]
                    ps = pspool.tile([128, Tc], F32, tag="ps", name=f"ps1_{c}_{f}")
                    for k in range(KD):
                        nc.tensor.matmul(
                            ps[:],
                            w1_t[g][:, fi, k, :],
                            xt_c[c][:, k, :],
                            start=(k == 0),
                            stop=(k == KD - 1),
                        )
                    h = hpool.tile([128, Tc], BF16, tag="h", name=f"h_{c}_{f}")
                    nc.scalar.activation(
                        h[:], ps[:], mybir.ActivationFunctionType.Relu,
                        bias=b1_t[:, f:f + 1], scale=1.0,
                    )
                    h_tiles.append(h)
                # y[d,t] = b2[d] + sum_f w2[f,d] * h[f,t]
                for d in range(KD):
                    ps2 = pspool.tile([128, Tc], F32, tag="ps", name=f"ps2_{c}_{d}")
                    for f in range(KF):
                        nc.tensor.matmul(
                            ps2[:],
                            w2_t[d][:, f, :],
                            h_tiles[f][:],
                            start=(f == 0),
                            stop=(f == KF - 1),
                        )
                    yo = ypool.tile([128, Tc], F32, tag="y", name=f"y_{c}_{d}")
                    nc.scalar.activation(
                        yo[:], ps2[:], mybir.ActivationFunctionType.Identity,
                        bias=b2_t[:, d:d + 1], scale=1.0,
                    )
                    nc.scalar.dma_start(out=yt[:, d, tok], in_=yo[:])
    nc.finalize()
    return nc


def _chunking(Tmax: int):
    """Balanced chunk sizes (multiples of 8, each <=512) covering Tmax."""
    Tmax = max(Tmax, 16)
    nch = max(1, math.ceil(Tmax / 512))
    units = math.ceil(Tmax / 8)
    per, extra = divmod(units, nch)
    chunks = tuple(8 * (per + (1 if i < extra else 0)) for i in range(nch))
    return chunks


def route_host(hidden_states, w_router):
    """fp64 router: logits, top-1 index, top-1 softmax prob."""
    hs_flat = np.asarray(hidden_states, np.float64).reshape(-1, D)
    logits = hs_flat @ np.asarray(w_router, np.float64)
    eidx = logits.argmax(1).astype(np.int32)
    ex = np.exp(logits - logits.max(1, keepdims=True))
    ptop = ex.max(1) / ex.sum(1)
    return logits, eidx, ptop


def pack_core_inputs(hs_flat32, tok, w1e, b1e, w2e, b2e, Tpad):
    """Pack one expert's tokens + weights into the device tile layouts."""
    n_e = len(tok)
    xt_h = np.zeros((128, KD, Tpad), BF16_NP)
    if n_e:
        xe = hs_flat32[tok]  # [n_e, D]
        xt_h[:, :, :n_e] = np.ascontiguousarray(
            xe.T.reshape(KD, 128, n_e).transpose(1, 0, 2)
        ).astype(BF16_NP)
    # w1 [D,F] -> [p, f, k, c] (f-tile-major columns), flattened per partition
    w1h = (
        w1e.reshape(KD, 128, KF, 128)
        .transpose(1, 2, 0, 3)
        .reshape(128, KF * KD * 128)
        .astype(BF16_NP)
    )
    # w2 [F,D] -> [p, d, kf, c] (d-tile-major columns)
    w2h = (
        w2e.reshape(KF, 128, KD, 128)
        .transpose(1, 2, 0, 3)
        .reshape(128, KD * KF * 128)
        .astype(BF16_NP)
    )
    b1h = np.ascontiguousarray(b1e.reshape(KF, 128).T).astype(np.float32)
    b2h = np.ascontiguousarray(b2e.reshape(KD, 128).T).astype(np.float32)
    return {"xt": xt_h, "w1": w1h, "w2": w2h, "b1": b1h, "b2": b2h}


def kernel(hidden_states, w_router, w1, b1, w2, b2):
    global last_results
    hs_flat32 = np.asarray(hidden_states, np.float32).reshape(-1, D)

    logits64, eidx, ptop = route_host(hidden_states, w_router)
    router_logits = logits64.astype(np.float32).reshape(B, S, E)

    counts = np.bincount(eidx, minlength=E)
    order = np.argsort(eidx, kind="stable")
    bounds = np.concatenate([[0], np.cumsum(counts)])
    chunks = _chunking(int(counts.max()))
    Tpad = sum(chunks)

    if chunks not in _build_cache:
        _build_cache[chunks] = build_bass(chunks)
    nc = _build_cache[chunks]

    w1 = np.asarray(w1, np.float32)
    w2 = np.asarray(w2, np.float32)
    b1 = np.asarray(b1, np.float32)
    b2 = np.asarray(b2, np.float32)
    toks = [order[bounds[e]:bounds[e + 1]] for e in range(E)]
    in_maps = [
        pack_core_inputs(hs_flat32, toks[e], w1[e], b1[e], w2[e], b2[e], Tpad)
        for e in range(E)
    ]

    trace = os.environ.get("BASS_KERNEL_TRACE") == "1"
    try:
        last_results = run_bass_kernel_spmd(
            nc, in_maps, core_ids=list(range(N_CORES)), trace=trace,
        )
    except Exception:
        # transient device/runtime hiccups happen on the shared machine;
        # one retry before giving up
        last_results = run_bass_kernel_spmd(
            nc, in_maps, core_ids=list(range(N_CORES)), trace=trace,
        )

    out_flat = np.zeros((B * S, D), np.float32)
    for e in range(E):
        n_e = int(counts[e])
        if n_e == 0:
            continue
        yt_r = last_results.results[e]["yt"]  # [128, KD, Tpad] f32
        ye = yt_r.transpose(1, 0, 2).reshape(D, Tpad)[:, :n_e].T  # [n_e, D]
        out_flat[toks[e]] = ptop[toks[e], None].astype(np.float32) * ye

    return (
        out_flat.reshape(B, S, D),
        router_logits,
        eidx.reshape(B, S),
    )
